# revision 2
# baseline (speedup 1.0000x reference)
"""VQ codebook-lookup (AudioQuantizer) Trainium2 kernel.

Problem: x [B=8, S=2048, D=512] f32, codebook [K=8192, D=512] f32.
  quantized[b,s] = codebook[argmin_k ||x[b,s] - codebook[k]||^2]

Sharding: data-parallel over batch — core b handles x[b] (2048 tokens),
codebook replicated on every core.

Per-core algorithm:
  argmin_k ||x-c_k||^2 == argmax_k (x.c_k - ||c_k||^2/2)
  1. scores = x @ c.T + n  computed on the PE with fp16 hi/lo splitting:
       x.c ~= xh.ch + xh.cl + xl.ch   (3 fp16 matmuls, exact products,
                                       fp32 PSUM accumulate -> fp32-class
                                       accuracy at 3/4 the cost of native
                                       fp32's 4-pass matmul)
     the norm term n_k = -||c_k||^2/2 is injected by one extra K=3 matmul
     of a ones-vector against 3 stacked fp16 rows (nh, nl, nll) derived
     from float64 host-side norms.
  2. Per 128-token tile: DVE Max/MaxIndex over the [128, 8192] fp32 score
     row gives the top-8 values/indices per token; index[0] is the argmax.
  3. GPSIMD indirect DMA gathers codebook[idx] rows; HWDGE stores them out.
"""

import numpy as np

P = 128          # partitions / token-tile size
NTILE = 512      # matmul moving free dim (one PSUM bank of fp32)

_PROGRAM_CACHE = {}


def _build_program(n_tok, K, D):
    import concourse.bacc as bacc
    import concourse.bass as bass
    import concourse.mybir as mybir
    import concourse.tile as tile

    TT = n_tok // P
    CT = K // NTILE
    DC = D // P
    f16 = mybir.dt.float16
    f32 = mybir.dt.float32
    u32 = mybir.dt.uint32

    nc = bacc.Bacc(
        "TRN2", target_bir_lowering=False, debug=False, enable_asserts=False
    )

    xh_d = nc.dram_tensor("xh", [P, DC, n_tok], f16, kind="ExternalInput").ap()
    xl_d = nc.dram_tensor("xl", [P, DC, n_tok], f16, kind="ExternalInput").ap()
    ch_d = nc.dram_tensor("ch", [P, DC, K], f16, kind="ExternalInput").ap()
    cl_d = nc.dram_tensor("cl", [P, DC, K], f16, kind="ExternalInput").ap()
    nrm_d = nc.dram_tensor("nrm", [3, K], f16, kind="ExternalInput").ap()
    cb_d = nc.dram_tensor("cb", [K, D], f32, kind="ExternalInput").ap()
    out_d = nc.dram_tensor("out", [n_tok, D], f32, kind="ExternalOutput").ap()

    with tile.TileContext(nc) as tc:
        with (
            tc.tile_pool(name="cpool", bufs=1) as cpool,
            tc.tile_pool(name="xpool", bufs=2) as xpool,
            tc.tile_pool(name="spool", bufs=1) as spool,
            tc.tile_pool(name="qpool", bufs=2) as qpool,
            tc.tile_pool(name="tpool", bufs=2) as tpool,
            tc.tile_pool(name="ppool", bufs=4, space="PSUM") as ppool,
        ):
            # Codebook halves + norm rows + ones: resident for the whole kernel
            ch_t = cpool.tile([P, DC, K], f16)
            cl_t = cpool.tile([P, DC, K], f16)
            nrm_t = cpool.tile([3, K], f16)
            ones_t = cpool.tile([3, P], f16)
            nc.sync.dma_start(ch_t[:], ch_d[:])
            nc.sync.dma_start(cl_t[:], cl_d[:])
            nc.sync.dma_start(nrm_t[:], nrm_d[:])
            nc.vector.memset(ones_t[:], 1.0)

            for tt in range(TT):
                tok = slice(tt * P, (tt + 1) * P)
                xh_t = xpool.tile([P, DC, P], f16, tag="xh")
                xl_t = xpool.tile([P, DC, P], f16, tag="xl")
                nc.sync.dma_start(xh_t[:], xh_d[:, :, tok])
                nc.sync.dma_start(xl_t[:], xl_d[:, :, tok])

                scores = spool.tile([P, K], f32)
                for ct in range(CT):
                    cs = slice(ct * NTILE, (ct + 1) * NTILE)
                    ps = ppool.tile([P, NTILE], f32)
                    for dc in range(DC):
                        nc.tensor.matmul(
                            ps[:], xh_t[:, dc, :], ch_t[:, dc, cs],
                            start=(dc == 0), stop=False,
                        )
                        nc.tensor.matmul(
                            ps[:], xh_t[:, dc, :], cl_t[:, dc, cs],
                            start=False, stop=False,
                        )
                        nc.tensor.matmul(
                            ps[:], xl_t[:, dc, :], ch_t[:, dc, cs],
                            start=False, stop=False,
                        )
                    nc.tensor.matmul(
                        ps[:], ones_t[:], nrm_t[:, cs], start=False, stop=True,
                    )
                    nc.scalar.copy(scores[:, cs], ps[:])

                vals = tpool.tile([P, 8], f32, tag="vals")
                idx = tpool.tile([P, 8], u32, tag="idx")
                nc.vector.max(out=vals[:], in_=scores[:])
                nc.vector.max_index(out=idx[:], in_max=vals[:], in_values=scores[:])

                q = qpool.tile([P, D], f32)
                nc.gpsimd.indirect_dma_start(
                    out=q[:],
                    out_offset=None,
                    in_=cb_d[:],
                    in_offset=bass.IndirectOffsetOnAxis(ap=idx[:, 0:1], axis=0),
                )
                nc.sync.dma_start(out_d[tok, :], q[:])

    nc.compile()
    return nc


def _host_prep(x_shard, codebook_prep):
    """Per-core input map. x_shard [n_tok, D] f32."""
    n_tok, D = x_shard.shape
    DC = D // P

    def to_chunks(a):  # [rows, D] -> [P, DC, rows] partition-major transpose
        return np.ascontiguousarray(
            a.T.reshape(DC, P, a.shape[0]).transpose(1, 0, 2)
        )

    xh = x_shard.astype(np.float16)
    xl = (x_shard - xh.astype(np.float32)).astype(np.float16)
    m = {"xh": to_chunks(xh), "xl": to_chunks(xl)}
    m.update(codebook_prep)
    return m


def _codebook_prep(codebook):
    K, D = codebook.shape
    ch = codebook.astype(np.float16)
    cl = (codebook - ch.astype(np.float32)).astype(np.float16)

    def to_chunks(a):
        return np.ascontiguousarray(
            a.T.reshape(D // P, P, K).transpose(1, 0, 2)
        )

    n64 = -0.5 * np.sum(codebook.astype(np.float64) ** 2, axis=1)
    nh = n64.astype(np.float16)
    r = n64 - nh.astype(np.float64)
    nl = r.astype(np.float16)
    nll = (r - nl.astype(np.float64)).astype(np.float16)
    nrm = np.ascontiguousarray(np.stack([nh, nl, nll], axis=0))
    return {
        "ch": to_chunks(ch),
        "cl": to_chunks(cl),
        "nrm": nrm,
        "cb": np.ascontiguousarray(codebook, dtype=np.float32),
    }


def kernel(x, codebook):
    from concourse import bass_utils

    x = np.asarray(x, dtype=np.float32)
    codebook = np.asarray(codebook, dtype=np.float32)
    B, S, D = x.shape
    K = codebook.shape[0]
    n_cores = 8
    assert B % n_cores == 0
    n_tok = (B // n_cores) * S

    key = (n_tok, K, D)
    if key not in _PROGRAM_CACHE:
        _PROGRAM_CACHE[key] = _build_program(n_tok, K, D)
    nc = _PROGRAM_CACHE[key]

    cb_prep = _codebook_prep(codebook)
    xs = x.reshape(n_cores, n_tok, D)
    in_maps = [_host_prep(xs[b], cb_prep) for b in range(n_cores)]

    res = bass_utils.run_bass_kernel_spmd(nc, in_maps, core_ids=list(range(n_cores)))
    out = np.stack([r["out"] for r in res.results], axis=0)
    return out.reshape(B, S, D).astype(np.float32)


if __name__ == "__main__":
    # Small end-to-end smoke test vs numpy reference
    rng = np.random.default_rng(0)
    x = rng.standard_normal((8, 128, 512)).astype(np.float32)
    cb = rng.standard_normal((1024, 512)).astype(np.float32)
    got = kernel(x, cb)
    flat = x.reshape(-1, 512)
    d = (
        np.sum(flat * flat, 1, keepdims=True)
        - 2.0 * flat @ cb.T
        + np.sum(cb * cb, 1)
    )
    want = cb[np.argmin(d, 1)].reshape(x.shape)
    err = np.abs(got - want)
    denom = np.abs(want).max()
    n_bad_rows = int((err.reshape(-1, 512).max(1) > 1e-3).sum())
    print("shape", got.shape, "max_abs_err", err.max(), "rel", err.max() / denom,
          "bad_rows", n_bad_rows, "/", flat.shape[0])
